# revision 1
# baseline (speedup 1.0000x reference)
"""CTC loss on 8 Trainium2 cores.

Strategy (data-parallel over batch, B=64 -> 8 utterances/core):
  Device per core:
    - Stream acts [3200, 5000] f32 once: ScalarE exp with accum_out -> Z[t,u]
      (memory-bound part, ~64MB/core).
    - CTC DP in rescaled linear space, layout [101 partitions (ext states),
      8 free (utterances)]; shifts are partition-offset reads. Skip-path mask
      folded into a pre-masked alpha copy; q = exp(gtilde) from a host-gathered,
      max-normalized emission tensor (exp'd on device). Exact rescale every 16
      steps via PE partition-sum + outer-product broadcast; log c accumulated.
    - Freeze (t >= input_len) and final readout are one-hot q columns.
  Host: tiny index prep (ext labels, masks, gather of 101 columns), final
  corrections sum(gmax) - sum(logZ) and mean.
"""
import os

import numpy as np

import concourse.bass as bass
import concourse.bacc as bacc
import concourse.mybir as mybir
import concourse.tile as tile
from concourse.bass_utils import run_bass_kernel_spmd

T, B, V, L = 400, 64, 5000, 50
S = 2 * L + 1            # 101
NCORES = 8
BS = B // NCORES         # 8
ROWS = T * BS            # 3200
P = 128
NT = ROWS // P           # 25
BOOST = np.float32(2.5)
K_RES = 16
NEG = np.float32(-10000.0)
F32 = mybir.dt.float32
AF = mybir.ActivationFunctionType
ALU = mybir.AluOpType
GCOLS = (T + 1) * 2 * BS  # 6416


def _build_program(T_steps=T, nt=NT, phases='all', reps=1):
    nc = bacc.Bacc(None, target_bir_lowering=False)
    rows = nt * P
    gcols = (T_steps + 1) * 2 * BS
    acts = nc.dram_tensor("acts", [rows, V], F32, kind="ExternalInput")
    g2 = nc.dram_tensor("g2", [S, gcols], F32, kind="ExternalInput")
    wmask = nc.dram_tensor("wmask", [P, nt], F32, kind="ExternalInput")
    sel = nc.dram_tensor("sel", [P, BS], F32, kind="ExternalInput")
    w1d = nc.dram_tensor("w1", [S, S], F32, kind="ExternalInput")
    w2d = nc.dram_tensor("w2", [S, S], F32, kind="ExternalInput")
    out_ll = nc.dram_tensor("out_ll", [1, BS], F32, kind="ExternalOutput")
    out_slz = nc.dram_tensor("out_slz", [BS, 1], F32, kind="ExternalOutput")

    with tile.TileContext(nc) as tc:
        with (
            tc.tile_pool(name="mp", bufs=1) as mp,
            tc.tile_pool(name="sp", bufs=3) as sp,
            tc.tile_pool(name="ep", bufs=2) as ep,
            tc.tile_pool(name="dp", bufs=2) as dpp,
            tc.tile_pool(name="pp", bufs=2, space="PSUM") as pp,
        ):
            for _rep in range(reps):
                # ---------------- constants / small inputs ----------------
                ones_col0 = mp.tile([S, 1], F32)
                nc.gpsimd.memset(ones_col0[:], 1.0)
                ones_row0 = mp.tile([1, S], F32)
                nc.gpsimd.memset(ones_row0[:], 1.0)
                selt0 = mp.tile([P, BS], F32)
                nc.gpsimd.dma_start(selt0[:], sel[:])
                # matmul operands funneled through DVE so each matmul carries a
                # single wait condition (PE LW has few sync-wait slots)
                ones_col = mp.tile([S, 1], F32)
                nc.vector.tensor_copy(ones_col[:], ones_col0[:])
                ones_row = mp.tile([1, S], F32)
                nc.vector.tensor_copy(ones_row[:], ones_row0[:])
                selt = mp.tile([P, BS], F32)
                nc.vector.tensor_copy(selt[:], selt0[:])
                wmt = mp.tile([P, nt], F32)
                nc.gpsimd.dma_start(wmt[:], wmask[:])

                gsb = mp.tile([S, gcols], F32)
                nc.gpsimd.dma_start(gsb[:], g2[:])
                q2 = mp.tile([S, gcols], F32)
                nc.scalar.activation(q2[:], gsb[:], AF.Exp)

                # ---------------- streaming logZ phase ----------------
                do_stream = phases in ('all', 'stream')
                do_dp = phases in ('all', 'dp', 'dpnr')
                zbuf = mp.tile([P, nt], F32)
                if not do_stream:
                    nc.gpsimd.memset(zbuf[:], 1.0)
                for k in (range(nt) if do_stream else []):
                    at = sp.tile([P, V], F32, tag="acts")
                    nc.gpsimd.dma_start(at[:], acts[k * P:(k + 1) * P, :])
                    nc.scalar.activation(at[:], at[:], AF.Exp,
                                         accum_out=zbuf[:, k:k + 1])
                lzbuf = mp.tile([P, nt], F32)
                nc.scalar.activation(lzbuf[:], zbuf[:], AF.Ln)
                wl = mp.tile([P, nt], F32)
                wpart = mp.tile([P, 1], F32)
                nc.vector.tensor_mul(wl[:], lzbuf[:], wmt[:])
                nc.vector.tensor_reduce(wpart[:], wl[:], axis=mybir.AxisListType.X,
                                        op=ALU.add)
                psz = pp.tile([BS, 1], F32, tag="psz")
                nc.tensor.matmul(psz[:], selt[:], wpart[:], start=True, stop=True)
                szout = mp.tile([BS, 1], F32)
                nc.vector.tensor_copy(szout[:], psz[:])
                nc.gpsimd.dma_start(out_slz[:], szout[:])

                # ---------------- DP phase ----------------
                # State w = [alpha | abar] in SBUF [101, 16] (base partition 0).
                # Shifts run on PE: psum = (I+Sh1)^T.T @ alpha + Sh2^T.T @ abar,
                # i.e. psum[s] = alpha[s] + alpha[s-1] + abar[s-2].
                w1t0 = mp.tile([S, S], F32)
                nc.gpsimd.dma_start(w1t0[:], w1d[:])
                w2t0 = mp.tile([S, S], F32)
                nc.gpsimd.dma_start(w2t0[:], w2d[:])
                w1t = mp.tile([S, S], F32)
                nc.vector.tensor_copy(w1t[:], w1t0[:])
                w2t = mp.tile([S, S], F32)
                nc.vector.tensor_copy(w2t[:], w2t0[:])

                aA = mp.tile([S, 2 * BS], F32)
                aB = mp.tile([S, 2 * BS], F32)
                llacc = mp.tile([1, BS], F32)
                nc.gpsimd.memset(llacc[:], 0.0)

                # init: alpha0 = q[0], abar0 = qbar[0]
                nc.vector.tensor_copy(aA[:], q2[:, 0:2 * BS])

                bufs = [aA, aB]
                cur = 0
                for t in (range(1, T_steps + 1) if do_dp else []):
                    src = bufs[cur]
                    dst = bufs[1 - cur]
                    ps = pp.tile([S, BS], F32, tag="ps")
                    nc.tensor.matmul(ps[:], w1t[:], src[:, 0:BS],
                                     start=True, stop=False)
                    nc.tensor.matmul(ps[:], w2t[:], src[:, BS:2 * BS],
                                     start=False, stop=True)
                    q0 = q2[:, 2 * BS * t:2 * BS * t + BS]
                    q1 = q2[:, 2 * BS * t + BS:2 * BS * (t + 1)]
                    nc.vector.tensor_mul(dst[:, 0:BS], q0, ps[:])
                    nc.vector.tensor_mul(dst[:, BS:2 * BS], q1, ps[:])
                    cur = 1 - cur
                    if t % K_RES == 0 and phases != 'dpnr':
                        src2 = bufs[cur]       # holds current state
                        dst2 = bufs[1 - cur]   # free buffer
                        csum = pp.tile([1, BS], F32, tag="csum")
                        nc.tensor.matmul(csum[:], ones_col[:], src2[:, 0:BS],
                                         start=True, stop=True)
                        r = dpp.tile([1, BS], F32, tag="r")
                        nc.vector.reciprocal(r[:], csum[:])
                        rb = pp.tile([S, BS], F32, tag="rb")
                        nc.tensor.matmul(rb[:], ones_row[:], r[:],
                                         start=True, stop=True)
                        nc.vector.tensor_mul(dst2[:, 0:BS],
                                             src2[:, 0:BS], rb[:])
                        nc.vector.tensor_mul(dst2[:, BS:2 * BS],
                                             src2[:, BS:2 * BS], rb[:])
                        # ll accumulation (off critical path)
                        lc = dpp.tile([1, BS], F32, tag="lc")
                        nc.scalar.activation(lc[:], csum[:], AF.Ln)
                        nc.vector.tensor_add(llacc[:], llacc[:], lc[:])
                        cur = 1 - cur

                nc.gpsimd.dma_start(out_ll[:], llacc[:])
    nc.compile()
    return nc


_PROGRAM = None
_LAST_RESULTS = None


def _get_program(reps=1, phases='all'):
    global _PROGRAM
    if _PROGRAM is None:
        _PROGRAM = {}
    key = (reps, phases)
    if key not in _PROGRAM:
        _PROGRAM[key] = _build_program(reps=reps, phases=phases)
    return _PROGRAM[key]


def _host_prep(acts, ilen, labels, llen):
    """Returns per-core input maps plus host-side correction sums."""
    Bb = acts.shape[1]
    ext = np.zeros((Bb, S), np.int32)
    ext[:, 1::2] = labels
    m = np.zeros((Bb, S), np.float32)
    m[:, 2:] = ((ext[:, 2:] != 0) & (ext[:, 2:] != ext[:, :-2])).astype(
        np.float32)
    mtil = np.zeros((Bb, S), np.float32)
    mtil[:, :S - 2] = m[:, 2:]
    logm = np.where(mtil > 0, np.float32(0.0), NEG)        # [B,S]

    g = np.take_along_axis(acts, np.broadcast_to(ext[None], (T, Bb, S)), axis=2)
    gmax = g.max(axis=2).astype(np.float32) - BOOST        # [T,B]
    gt = (g - gmax[:, :, None]).astype(np.float32)         # [T,B,S]

    srange = np.arange(S)
    valid_s = srange[None, :] < (2 * llen + 1)[:, None]    # [B,S]
    gt = np.where(valid_s[None], gt, NEG)
    onehot = np.where(srange[None, :] == (2 * llen)[:, None],
                      np.float32(0.0), NEG)                # [B,S]
    tmask = np.arange(T)[:, None] < ilen[None, :]          # [T,B]
    gt = np.where(tmask[:, :, None], gt, onehot[None])
    gt[0, :, 2:] = NEG                                     # init: s in {0,1}

    gt_all = np.concatenate([gt, onehot[None]], axis=0)    # [T+1,B,S]
    g2 = np.stack([gt_all, gt_all + logm[None]], axis=1)   # [T+1,2,B,S]
    g2 = np.maximum(g2, NEG).astype(np.float32)

    sum_gmax = (gmax.astype(np.float64) * tmask).sum(axis=0)  # [B]

    in_maps = []
    for c in range(NCORES):
        cs = slice(c * BS, (c + 1) * BS)
        acts_c = np.ascontiguousarray(
            acts[:, cs, :].reshape(ROWS, V).astype(np.float32))
        g2_c = np.ascontiguousarray(
            g2[:, :, cs, :].transpose(3, 0, 1, 2).reshape(S, GCOLS)
            .astype(np.float32))
        wm_c = np.ascontiguousarray(
            tmask[:, cs].astype(np.float32).reshape(ROWS).reshape(NT, P).T)
        sel_c = (np.arange(P)[:, None] % BS ==
                 np.arange(BS)[None, :]).astype(np.float32)
        w1 = (np.eye(S) + np.eye(S, k=1)).astype(np.float32)   # lhsT: I+Sh1
        w2 = np.eye(S, k=2).astype(np.float32)                  # lhsT: Sh2
        in_maps.append({"acts": acts_c, "g2": g2_c, "wmask": wm_c,
                       "sel": sel_c, "w1": w1, "w2": w2})
    return in_maps, sum_gmax


def kernel(activations, input_lengths, labels, label_lengths):
    acts = np.ascontiguousarray(np.asarray(activations, dtype=np.float32))
    ilen = np.asarray(input_lengths, dtype=np.int32)
    labs = np.asarray(labels, dtype=np.int32)
    llen = np.asarray(label_lengths, dtype=np.int32)

    in_maps, sum_gmax = _host_prep(acts, ilen, labs, llen)
    nc = _get_program(reps=int(os.environ.get("CTC_REPS", "1")), phases=os.environ.get("CTC_PHASES", "all"))
    _r = run_bass_kernel_spmd(nc, in_maps, list(range(NCORES)))
    global _LAST_RESULTS
    _LAST_RESULTS = _r
    res = _r.results

    losses = np.zeros(B, np.float64)
    for c in range(NCORES):
        ll = res[c]["out_ll"].reshape(BS).astype(np.float64)
        slz = res[c]["out_slz"].reshape(BS).astype(np.float64)
        cs = slice(c * BS, (c + 1) * BS)
        losses[cs] = -(ll + sum_gmax[cs] - slz)
    return np.float32(losses.mean())



# revision 3
# speedup vs baseline: 2.4701x; 2.4701x over previous
"""CTC loss on 8 Trainium2 cores.

Strategy (data-parallel over batch, B=64 -> 8 utterances/core):
  Device per core, two concurrent pipelines:
    - Streaming (ACT + HWDGE-DMA): acts [3200, 5000] f32 in 13 big tiles
      [128, 10000]; ScalarE exp with accum_out -> Z per (t,u) row. ~64MB,
      memory-bound.
    - CTC DP on DVE only, layout [8 utterances (partitions), 101 ext states
      (free)]: shifts are free-axis offset views with zero guard columns;
      per step 3 DVE ops (add, dup-add, mul) in bf16 (2x DVE mode). Skip
      mask folded into a second emission table q1 = q0*mask; emissions
      q = exp(gtilde) precomputed host-side in bf16, streamed in 9 chunks
      on the sync-engine HWDGE ring (separate from the acts ring).
      Rescale by sum every 16 steps via scalar_tensor_tensor with a
      per-partition reciprocal; log c accumulated on-device (ln at end).
    - Freeze (t >= input_len) and final readout folded into one-hot q
      columns; final step's STT accum gives the end-state mass directly.
  Host: index prep (ext labels, masks, gather of 101 columns, exp),
  final corrections sum(gmax) - sum(logZ) and mean.
"""
import numpy as np
import ml_dtypes

import concourse.bass as bass
import concourse.bacc as bacc
import concourse.mybir as mybir
import concourse.tile as tile
from concourse.bass_utils import run_bass_kernel_spmd

T, B, V, L = 400, 64, 5000, 50
S = 2 * L + 1            # 101
NCORES = 8
BS = B // NCORES         # 8
ROWS = T * BS            # 3200
NT = 12                  # full [128, 2, V] tiles; + one [128, V] tail tile
NZ = 2 * NT + 1          # 25 Z slots
BOOST = np.float32(2.5)
K_RES = 16
NRES = T // K_RES        # 25 rescales; cs has NRES+1 slots
CH = 50                  # DP steps per q chunk
QW = 2 * S               # 202
QCOLS = (T + 1) * QW     # 81002
NCH = (T + 1 + CH - 1) // CH  # 9
NEG = np.float32(-10000.0)
F32 = mybir.dt.float32
BF16 = mybir.dt.bfloat16
bf16 = ml_dtypes.bfloat16
AF = mybir.ActivationFunctionType
ALU = mybir.AluOpType


def _build_program():
    nc = bacc.Bacc(None, target_bir_lowering=False)
    acts = nc.dram_tensor("acts", [ROWS, V], F32, kind="ExternalInput")
    qq = nc.dram_tensor("qq", [BS, QCOLS], BF16, kind="ExternalInput")
    wm = nc.dram_tensor("wm", [128, NZ], F32, kind="ExternalInput")
    sel = nc.dram_tensor("sel", [128, BS], F32, kind="ExternalInput")
    out_ll = nc.dram_tensor("out_ll", [BS, 1], F32, kind="ExternalOutput")
    out_slz = nc.dram_tensor("out_slz", [BS, 1], F32, kind="ExternalOutput")

    with tile.TileContext(nc) as tc:
        with (
            tc.tile_pool(name="mp", bufs=1) as mp,
            tc.tile_pool(name="ap", bufs=3) as ap,
            tc.tile_pool(name="qp", bufs=3) as qp,
            tc.tile_pool(name="pp", bufs=1, space="PSUM") as pp,
        ):
            # ---------------- persistent state ----------------
            ws = mp.tile([BS, 2 * S + 4], BF16)   # [g g a(101) g g b(101)]
            nc.vector.memset(ws[:], 0.0)
            t1 = mp.tile([BS, S], BF16)
            t2 = mp.tile([BS, 2 * S], BF16)
            cs = mp.tile([BS, NRES + 1], F32)
            rbuf = mp.tile([BS, 1], F32)
            zbuf = mp.tile([128, NZ], F32)
            wmt = mp.tile([128, NZ], F32)
            selt = mp.tile([128, BS], F32)
            selc = mp.tile([128, BS], F32)

            def wsb():   # state write view: cols {2..102, 105..205}
                return ws[:].rearrange("p (b s) -> p b s", b=2)[:, :, 2:S + 2]

            def t2v():
                return t2[:].rearrange("p (b s) -> p b s", b=2)

            def t1b():
                return t1[:].unsqueeze(1).broadcast_to((BS, 2, S))

            def bsh2():  # beta shifted by 2: cols 103..203 (2 guard zeros)
                return ws[:, S + 2:2 * S + 2].unsqueeze(1).broadcast_to(
                    (BS, 2, S))

            # ---------------- streaming pipeline (ACT engine) ----------
            for k in range(NT + 1):
                at = ap.tile([128, 2 * V], F32, tag="acts")
                if k < NT:
                    src = acts[256 * k:256 * (k + 1), :].rearrange(
                        "(b p) v -> p b v", b=2)
                    nc.scalar.dma_start(at[:], src)
                    for bb in range(2):
                        blk = at[:, bb * V:(bb + 1) * V]
                        nc.scalar.activation(blk, blk, AF.Exp,
                                             accum_out=zbuf[:, 2 * k + bb:
                                                            2 * k + bb + 1])
                else:
                    nc.scalar.dma_start(at[:, 0:V], acts[256 * NT:ROWS, :])
                    blk = at[:, 0:V]
                    nc.scalar.activation(blk, blk, AF.Exp,
                                         accum_out=zbuf[:, 2 * NT:2 * NT + 1])

            # ---------------- DP pipeline (DVE + sync-ring DMA) ---------
            for c in range(NCH):
                c0 = c * CH * QW
                ncols = min(QCOLS - c0, CH * QW)
                qt = qp.tile([BS, CH * QW], BF16, tag="q")
                nc.sync.dma_start(qt[:, 0:ncols], qq[:, c0:c0 + ncols])
                if c == 0:
                    nc.vector.tensor_copy(
                        wsb(), qt[:, 0:QW].rearrange("p (b s) -> p b s", b=2))
                    trange = range(1, CH)
                else:
                    trange = range(CH * c, min(CH * (c + 1), T + 1))
                for t in trange:
                    base = (t - CH * c) * QW
                    q01 = qt[:, base:base + QW].rearrange(
                        "p (b s) -> p b s", b=2)
                    nc.vector.tensor_add(t1[:], ws[:, 2:S + 2],
                                         ws[:, 1:S + 1])
                    nc.vector.tensor_add(t2v(), t1b(), bsh2())
                    if t % K_RES == K_RES - 1:
                        j = t // K_RES
                        nc.vector.scalar_tensor_tensor(
                            wsb(), t2v(), 1.0, q01, ALU.mult, ALU.mult,
                            accum_out=cs[:, j:j + 1])
                        nc.vector.reciprocal(rbuf[:], cs[:, j:j + 1])
                    elif t % K_RES == 0:
                        acc = cs[:, NRES:NRES + 1] if t == T else None
                        nc.vector.scalar_tensor_tensor(
                            wsb(), t2v(), rbuf[:, 0:1], q01, ALU.mult,
                            ALU.mult, accum_out=acc)
                    else:
                        nc.vector.tensor_mul(wsb(), t2v(), q01)

            # small inputs for the tail (issued after q chunks on SP ring)
            nc.sync.dma_start(wmt[:], wm[:])
            nc.sync.dma_start(selt[:], sel[:])
            nc.vector.tensor_copy(selc[:], selt[:])

            # ---------------- tail: logZ reduction + loss ----------------
            lnz = mp.tile([128, NZ], F32)
            nc.scalar.activation(lnz[:], zbuf[:], AF.Ln)
            wl = mp.tile([128, NZ], F32)
            nc.vector.tensor_mul(wl[:], lnz[:], wmt[:])
            wpart = mp.tile([128, 1], F32)
            nc.vector.tensor_reduce(wpart[:], wl[:], axis=mybir.AxisListType.X,
                                    op=ALU.add)
            psz = pp.tile([BS, 1], F32, tag="psz")
            nc.tensor.matmul(psz[:], selc[:], wpart[:], start=True, stop=True)
            szout = mp.tile([BS, 1], F32)
            nc.vector.tensor_copy(szout[:], psz[:])
            nc.sync.dma_start(out_slz[:], szout[:])

            lncs = mp.tile([BS, NRES + 1], F32)
            nc.scalar.activation(lncs[:], cs[:], AF.Ln)
            llp = mp.tile([BS, 1], F32)
            nc.vector.tensor_reduce(llp[:], lncs[:], axis=mybir.AxisListType.X,
                                    op=ALU.add)
            nc.sync.dma_start(out_ll[:], llp[:])
    nc.compile()
    return nc


_PROGRAM = None
_LAST_RESULTS = None


def _get_program():
    global _PROGRAM
    if _PROGRAM is None:
        _PROGRAM = _build_program()
    return _PROGRAM


def _host_prep(acts, ilen, labels, llen):
    """Per-core input maps plus host-side correction sums."""
    ext = np.zeros((B, S), np.int64)
    ext[:, 1::2] = labels
    m = np.zeros((B, S), np.float32)
    m[:, 2:] = ((ext[:, 2:] != 0) & (ext[:, 2:] != ext[:, :-2])).astype(
        np.float32)
    mtil = np.zeros((B, S), np.float32)
    mtil[:, :S - 2] = m[:, 2:]

    g = np.take_along_axis(acts, np.broadcast_to(ext[None], (T, B, S)), axis=2)
    gmax = g.max(axis=2).astype(np.float32) - BOOST          # [T, B]
    gt = (g - gmax[:, :, None]).astype(np.float32)           # [T, B, S]

    srange = np.arange(S)
    valid_s = srange[None, :] < (2 * llen + 1)[:, None]      # [B, S]
    gt = np.where(valid_s[None], gt, NEG)
    onehot = np.where(srange[None, :] == (2 * llen)[:, None],
                      np.float32(0.0), NEG)                  # [B, S]
    tmask = np.arange(T)[:, None] < ilen[None, :]            # [T, B]
    gt = np.where(tmask[:, :, None], gt, onehot[None])
    gt[0, :, 2:] = NEG
    gt_all = np.concatenate([gt, onehot[None]], axis=0)      # [T+1, B, S]

    q0 = np.exp(gt_all, dtype=np.float32)                    # [T+1, B, S]
    q1 = q0 * mtil[None]
    qarr = np.empty((B, T + 1, 2, S), np.float32)
    qarr[:, :, 0, :] = q0.transpose(1, 0, 2)
    qarr[:, :, 1, :] = q1.transpose(1, 0, 2)
    qq_full = qarr.reshape(B, QCOLS).astype(bf16)

    sum_gmax = (gmax.astype(np.float64) * tmask).sum(axis=0)  # [B]

    p = np.arange(128)
    rcols = np.empty((128, NZ), np.int64)
    for ccol in range(2 * NT):
        k, bb = ccol // 2, ccol % 2
        rcols[:, ccol] = 256 * k + 128 * bb + p
    rcols[:, 2 * NT] = 256 * NT + p
    tcols = rcols // BS                                      # [128, NZ]
    ucols = p % BS                                           # [128]
    sel_c = (p[:, None] % BS == np.arange(BS)[None, :]).astype(np.float32)

    in_maps = []
    for c in range(NCORES):
        csl = slice(c * BS, (c + 1) * BS)
        acts_c = np.ascontiguousarray(
            acts[:, csl, :].reshape(ROWS, V).astype(np.float32))
        qq_c = np.ascontiguousarray(qq_full[csl])
        wm_c = (tcols < ilen[csl][ucols][:, None]).astype(np.float32)
        in_maps.append({"acts": acts_c, "qq": qq_c, "wm": wm_c,
                        "sel": sel_c})
    return in_maps, sum_gmax


def kernel(activations, input_lengths, labels, label_lengths):
    acts = np.asarray(activations, dtype=np.float32)
    ilen = np.asarray(input_lengths, dtype=np.int64)
    labs = np.asarray(labels, dtype=np.int64)
    llen = np.asarray(label_lengths, dtype=np.int64)

    in_maps, sum_gmax = _host_prep(acts, ilen, labs, llen)
    nc = _get_program()
    _r = run_bass_kernel_spmd(nc, in_maps, list(range(NCORES)))
    global _LAST_RESULTS
    _LAST_RESULTS = _r
    res = _r.results

    losses = np.zeros(B, np.float64)
    for c in range(NCORES):
        ll = res[c]["out_ll"].reshape(BS).astype(np.float64)
        slz = res[c]["out_slz"].reshape(BS).astype(np.float64)
        csl = slice(c * BS, (c + 1) * BS)
        losses[csl] = -(ll + sum_gmax[csl] - slz)
    return np.float32(losses.mean())


# revision 7
# speedup vs baseline: 2.5628x; 1.0375x over previous
"""CTC loss on 8 Trainium2 cores.

Strategy (data-parallel over batch, B=64 -> 8 utterances/core):
  Device per core, two concurrent pipelines:
    - Streaming (ACT + HWDGE-DMA): acts [3200, 5000] f32 in 13 big tiles
      [128, 10000]; ScalarE exp with accum_out -> Z per (t,u) row. ~64MB,
      memory-bound.
    - CTC DP on DVE only, layout [8 utterances (partitions), 101 ext states
      (free)]: shifts are free-axis offset views with zero guard columns;
      per step 3 DVE ops (add, dup-add, mul) in bf16 (2x DVE mode). Skip
      mask folded into a second emission table q1 = q0*mask; emissions
      q = exp(gtilde) precomputed host-side in bf16, streamed in 9 chunks
      on the sync-engine HWDGE ring (separate from the acts ring).
      Rescale by sum every 16 steps via scalar_tensor_tensor with a
      per-partition reciprocal; log c accumulated on-device (ln at end).
    - Freeze (t >= input_len) and final readout folded into one-hot q
      columns; final step's STT accum gives the end-state mass directly.
  Host: index prep (ext labels, masks, gather of 101 columns, exp),
  final corrections sum(gmax) - sum(logZ) and mean.
"""
import numpy as np
import ml_dtypes

import concourse.bass as bass
import concourse.bacc as bacc
import concourse.mybir as mybir
import concourse.tile as tile
from concourse.bass_utils import run_bass_kernel_spmd

T, B, V, L = 400, 64, 5000, 50
S = 2 * L + 1            # 101
NCORES = 8
BS = B // NCORES         # 8
ROWS = T * BS            # 3200
NT = 12                  # full [128, 2, V] tiles; + one [128, V] tail tile
NZ = 2 * NT + 1          # 25 Z slots
BOOST = np.float32(2.5)
K_RES = 16
NRES = T // K_RES        # 25 rescales; cs has NRES+1 slots
CH = 50                  # DP steps per q chunk
QW = 2 * S               # 202
QCOLS = (T + 1) * QW     # 81002
NCH = (T + 1 + CH - 1) // CH  # 9
NEG = np.float32(-10000.0)
F32 = mybir.dt.float32
BF16 = mybir.dt.bfloat16
bf16 = ml_dtypes.bfloat16
AF = mybir.ActivationFunctionType
ALU = mybir.AluOpType


def _build_program():
    nc = bacc.Bacc(None, target_bir_lowering=False)
    acts = nc.dram_tensor("acts", [ROWS, V], F32, kind="ExternalInput")
    qq = nc.dram_tensor("qq", [BS, QCOLS], BF16, kind="ExternalInput")
    wm = nc.dram_tensor("wm", [128, NZ], F32, kind="ExternalInput")
    sel = nc.dram_tensor("sel", [128, BS], F32, kind="ExternalInput")
    out_ll = nc.dram_tensor("out_ll", [BS, 1], F32, kind="ExternalOutput")
    out_slz = nc.dram_tensor("out_slz", [BS, 1], F32, kind="ExternalOutput")

    with tile.TileContext(nc) as tc:
        with (
            tc.tile_pool(name="mp", bufs=1) as mp,
            tc.tile_pool(name="ap", bufs=3) as ap,
            tc.tile_pool(name="qp", bufs=3) as qp,
            tc.tile_pool(name="pp", bufs=1, space="PSUM") as pp,
        ):
            # ---------------- persistent state ----------------
            ws = mp.tile([BS, 2 * S + 4], BF16)   # [g g a(101) g g b(101)]
            nc.vector.memset(ws[:], 0.0)
            t1 = mp.tile([BS, S], BF16)
            t2 = mp.tile([BS, S], BF16)
            cs = mp.tile([BS, NRES + 1], F32)
            rbuf = mp.tile([BS, 1], F32)
            zbuf = mp.tile([128, NZ], F32)
            wmt = mp.tile([128, NZ], F32)
            selt = mp.tile([128, BS], F32)
            selc = mp.tile([128, BS], F32)

            def wsb():   # state write view: cols {2..102, 105..205}
                return ws[:].rearrange("p (b s) -> p b s", b=2)[:, :, 2:S + 2]

            def t2b():   # t2 broadcast over the two state blocks
                return t2[:].unsqueeze(1).broadcast_to((BS, 2, S))

            def bsh2():  # beta shifted by 2: cols 103..203 (2 guard zeros)
                return ws[:, S + 2:2 * S + 2]

            # ---------------- streaming pipeline (ACT engine) ----------
            # q chunk 0 rides the ACT HWDGE ring ahead of the 5MB acts
            # tiles, so the DP starts within a few us.
            qt0 = qp.tile([BS, CH * QW], BF16, tag="q")
            nc.scalar.dma_start(qt0[:], qq[:, 0:CH * QW])
            for k in range(NT + 1):
                at = ap.tile([128, 2 * V], F32, tag="acts")
                if k < NT:
                    src = acts[256 * k:256 * (k + 1), :].rearrange(
                        "(b p) v -> p b v", b=2)
                    nc.scalar.dma_start(at[:], src)
                    for bb in range(2):
                        blk = at[:, bb * V:(bb + 1) * V]
                        nc.scalar.activation(blk, blk, AF.Exp,
                                             accum_out=zbuf[:, 2 * k + bb:
                                                            2 * k + bb + 1])
                else:
                    nc.scalar.dma_start(at[:, 0:V], acts[256 * NT:ROWS, :])
                    blk = at[:, 0:V]
                    nc.scalar.activation(blk, blk, AF.Exp,
                                         accum_out=zbuf[:, 2 * NT:2 * NT + 1])

            # ---------------- DP pipeline (DVE + sync-ring DMA) ---------
            for c in range(NCH):
                c0 = c * CH * QW
                ncols = min(QCOLS - c0, CH * QW)
                if c == 0:
                    qt = qt0
                    nc.vector.tensor_copy(
                        wsb(), qt[:, 0:QW].rearrange("p (b s) -> p b s", b=2))
                    trange = range(1, CH)
                else:
                    qt = qp.tile([BS, CH * QW], BF16, tag="q")
                    nc.sync.dma_start(qt[:, 0:ncols], qq[:, c0:c0 + ncols])
                    trange = range(CH * c, min(CH * (c + 1), T + 1))
                for t in trange:
                    base = (t - CH * c) * QW
                    q01 = qt[:, base:base + QW].rearrange(
                        "p (b s) -> p b s", b=2)
                    nc.vector.tensor_add(t1[:], ws[:, 2:S + 2],
                                         ws[:, 1:S + 1])
                    nc.vector.tensor_add(t2[:], t1[:], bsh2())
                    if t % K_RES == K_RES - 1:
                        j = t // K_RES
                        nc.vector.scalar_tensor_tensor(
                            wsb(), t2b(), 1.0, q01, ALU.mult, ALU.mult,
                            accum_out=cs[:, j:j + 1])
                        nc.vector.reciprocal(rbuf[:], cs[:, j:j + 1])
                    elif t % K_RES == 0:
                        acc = cs[:, NRES:NRES + 1] if t == T else None
                        nc.vector.scalar_tensor_tensor(
                            wsb(), t2b(), rbuf[:, 0:1], q01, ALU.mult,
                            ALU.mult, accum_out=acc)
                    else:
                        nc.vector.tensor_mul(wsb(), t2b(), q01)

            # small inputs for the tail (issued after q chunks on SP ring)
            nc.sync.dma_start(wmt[:], wm[:])
            nc.sync.dma_start(selt[:], sel[:])
            nc.vector.tensor_copy(selc[:], selt[:])

            # ---------------- tail: logZ reduction + loss ----------------
            lnz = mp.tile([128, NZ], F32)
            nc.scalar.activation(lnz[:], zbuf[:], AF.Ln)
            wl = mp.tile([128, NZ], F32)
            nc.vector.tensor_mul(wl[:], lnz[:], wmt[:])
            wpart = mp.tile([128, 1], F32)
            nc.vector.tensor_reduce(wpart[:], wl[:], axis=mybir.AxisListType.X,
                                    op=ALU.add)
            psz = pp.tile([BS, 1], F32, tag="psz")
            nc.tensor.matmul(psz[:], selc[:], wpart[:], start=True, stop=True)
            szout = mp.tile([BS, 1], F32)
            nc.vector.tensor_copy(szout[:], psz[:])
            nc.sync.dma_start(out_slz[:], szout[:])

            lncs = mp.tile([BS, NRES + 1], F32)
            nc.scalar.activation(lncs[:], cs[:], AF.Ln)
            llp = mp.tile([BS, 1], F32)
            nc.vector.tensor_reduce(llp[:], lncs[:], axis=mybir.AxisListType.X,
                                    op=ALU.add)
            nc.sync.dma_start(out_ll[:], llp[:])
    nc.compile()
    return nc


_PROGRAM = None
_LAST_RESULTS = None


def _get_program():
    global _PROGRAM
    if _PROGRAM is None:
        _PROGRAM = _build_program()
    return _PROGRAM


def _host_prep(acts, ilen, labels, llen):
    """Per-core input maps plus host-side correction sums."""
    ext = np.zeros((B, S), np.int64)
    ext[:, 1::2] = labels
    m = np.zeros((B, S), np.float32)
    m[:, 2:] = ((ext[:, 2:] != 0) & (ext[:, 2:] != ext[:, :-2])).astype(
        np.float32)
    mtil = np.zeros((B, S), np.float32)
    mtil[:, :S - 2] = m[:, 2:]

    g = np.take_along_axis(acts, np.broadcast_to(ext[None], (T, B, S)), axis=2)
    gmax = g.max(axis=2).astype(np.float32) - BOOST          # [T, B]
    gt = (g - gmax[:, :, None]).astype(np.float32)           # [T, B, S]

    srange = np.arange(S)
    valid_s = srange[None, :] < (2 * llen + 1)[:, None]      # [B, S]
    gt = np.where(valid_s[None], gt, NEG)
    onehot = np.where(srange[None, :] == (2 * llen)[:, None],
                      np.float32(0.0), NEG)                  # [B, S]
    tmask = np.arange(T)[:, None] < ilen[None, :]            # [T, B]
    gt = np.where(tmask[:, :, None], gt, onehot[None])
    gt[0, :, 2:] = NEG
    gt_all = np.concatenate([gt, onehot[None]], axis=0)      # [T+1, B, S]

    q0 = np.exp(gt_all, dtype=np.float32)                    # [T+1, B, S]
    q1 = q0 * mtil[None]
    qarr = np.empty((B, T + 1, 2, S), np.float32)
    qarr[:, :, 0, :] = q0.transpose(1, 0, 2)
    qarr[:, :, 1, :] = q1.transpose(1, 0, 2)
    qq_full = qarr.reshape(B, QCOLS).astype(bf16)

    sum_gmax = (gmax.astype(np.float64) * tmask).sum(axis=0)  # [B]

    p = np.arange(128)
    rcols = np.empty((128, NZ), np.int64)
    for ccol in range(2 * NT):
        k, bb = ccol // 2, ccol % 2
        rcols[:, ccol] = 256 * k + 128 * bb + p
    rcols[:, 2 * NT] = 256 * NT + p
    tcols = rcols // BS                                      # [128, NZ]
    ucols = p % BS                                           # [128]
    sel_c = (p[:, None] % BS == np.arange(BS)[None, :]).astype(np.float32)

    in_maps = []
    for c in range(NCORES):
        csl = slice(c * BS, (c + 1) * BS)
        acts_c = np.ascontiguousarray(
            acts[:, csl, :].reshape(ROWS, V).astype(np.float32))
        qq_c = np.ascontiguousarray(qq_full[csl])
        wm_c = (tcols < ilen[csl][ucols][:, None]).astype(np.float32)
        in_maps.append({"acts": acts_c, "qq": qq_c, "wm": wm_c,
                        "sel": sel_c})
    return in_maps, sum_gmax


def kernel(activations, input_lengths, labels, label_lengths):
    acts = np.asarray(activations, dtype=np.float32)
    ilen = np.asarray(input_lengths, dtype=np.int64)
    labs = np.asarray(labels, dtype=np.int64)
    llen = np.asarray(label_lengths, dtype=np.int64)

    in_maps, sum_gmax = _host_prep(acts, ilen, labs, llen)
    nc = _get_program()
    _r = run_bass_kernel_spmd(nc, in_maps, list(range(NCORES)))
    global _LAST_RESULTS
    _LAST_RESULTS = _r
    res = _r.results

    losses = np.zeros(B, np.float64)
    for c in range(NCORES):
        ll = res[c]["out_ll"].reshape(BS).astype(np.float64)
        slz = res[c]["out_slz"].reshape(BS).astype(np.float64)
        csl = slice(c * BS, (c + 1) * BS)
        losses[csl] = -(ll + sum_gmax[csl] - slz)
    return np.float32(losses.mean())


# revision 9
# speedup vs baseline: 2.6652x; 1.0400x over previous
"""CTC loss on 8 Trainium2 cores.

Strategy (data-parallel over batch, B=64 -> 8 utterances/core):
  Device per core, two concurrent pipelines:
    - Streaming (ACT + HWDGE-DMA): acts [3200, 5000] f32 in 13 big tiles
      [128, 2, 5000]; ScalarE exp with accum_out -> Z per (t,u) row.
      ~64MB/core, memory-bound.
    - CTC DP on DVE only, layout [8 utterances (partitions), 101 ext states
      (free)]: shifts are free-axis offset views with zero guard columns;
      per step 3 DVE ops (add, add, dup-mul via broadcast+2-block write)
      in bf16 (2x DVE mode). Skip mask folded into a second emission table
      q1 = q0*mask; emissions q = exp(gtilde) precomputed host-side in
      bf16, streamed in chunks (first two small chunks ride the ACT HWDGE
      ring ahead of the acts tiles for a fast DP start; the rest use the
      sync-engine ring). Rescale by sum every 16 steps via
      scalar_tensor_tensor with a per-partition reciprocal; the STT accum
      also logs the rescale constants c into cs.
    - Freeze (t >= input_len) and final readout folded into one-hot q
      columns; final step's STT accum gives the end-state mass directly.
  Device outputs are raw Z [128, 25] and cs [8, 26]; the host does the
  tiny ln + masked reductions, corrections sum(gmax) - sum(logZ), mean.
"""
import numpy as np
import ml_dtypes

import concourse.bass as bass
import concourse.bacc as bacc
import concourse.mybir as mybir
import concourse.tile as tile
from concourse.bass_utils import run_bass_kernel_spmd

T, B, V, L = 400, 64, 5000, 50
S = 2 * L + 1            # 101
NCORES = 8
BS = B // NCORES         # 8
ROWS = T * BS            # 3200
NT = 12                  # full [128, 2, V] tiles; + one [128, V] tail tile
NZ = 2 * NT + 1          # 25 Z slots
BOOST = np.float32(2.5)
K_RES = 16
NRES = T // K_RES        # 25 rescales; cs has NRES+1 slots
QW = 2 * S               # 202
QCOLS = (T + 1) * QW     # 81002
# q chunk boundaries in t: small first chunks for a fast DP start
CHB = [0, 5, 20, 50, 100, 150, 200, 250, 300, 350, T + 1]
CHMAX = max(b - a for a, b in zip(CHB, CHB[1:]))
NEG = np.float32(-10000.0)
F32 = mybir.dt.float32
BF16 = mybir.dt.bfloat16
bf16 = ml_dtypes.bfloat16
AF = mybir.ActivationFunctionType
ALU = mybir.AluOpType


def _build_program():
    nc = bacc.Bacc(None, target_bir_lowering=False)
    acts = nc.dram_tensor("acts", [ROWS, V], F32, kind="ExternalInput")
    qq = nc.dram_tensor("qq", [BS, QCOLS], BF16, kind="ExternalInput")
    out_z = nc.dram_tensor("out_z", [128, NZ], F32, kind="ExternalOutput")
    out_cs = nc.dram_tensor("out_cs", [BS, NRES + 1], F32,
                            kind="ExternalOutput")

    with tile.TileContext(nc) as tc:
        with (
            tc.tile_pool(name="mp", bufs=1) as mp,
            tc.tile_pool(name="ap", bufs=3) as ap,
            tc.tile_pool(name="qp", bufs=3) as qp,
        ):
            # ---------------- persistent state ----------------
            ws = mp.tile([BS, 2 * S + 4], BF16)   # [g g a(101) g g b(101)]
            nc.vector.memset(ws[:], 0.0)
            t1 = mp.tile([BS, S], BF16)
            t2 = mp.tile([BS, S], BF16)
            cs = mp.tile([BS, NRES + 1], F32)
            rbuf = mp.tile([BS, 1], F32)
            zbuf = mp.tile([128, NZ], F32)

            def wsb():   # state write view: cols {2..102, 105..205}
                return ws[:].rearrange("p (b s) -> p b s", b=2)[:, :, 2:S + 2]

            def t2b():   # t2 broadcast over the two state blocks
                return t2[:].unsqueeze(1).broadcast_to((BS, 2, S))

            # ------------- streaming pipeline (ACT engine + ring) -------
            # First two (small) q chunks ride the ACT HWDGE ring ahead of
            # the 5MB acts tiles, so the DP starts within a few us.
            qts = {}
            for c in range(2):
                qte = qp.tile([BS, CHMAX * QW], BF16, tag="q")
                qts[c] = qte
                nc.scalar.dma_start(
                    qte[:, 0:(CHB[c + 1] - CHB[c]) * QW],
                    qq[:, CHB[c] * QW:CHB[c + 1] * QW])
            for k in range(NT + 1):
                at = ap.tile([128, 2 * V], F32, tag="acts")
                if k < NT:
                    src = acts[256 * k:256 * (k + 1), :].rearrange(
                        "(b p) v -> p b v", b=2)
                    nc.scalar.dma_start(at[:], src)
                    for bb in range(2):
                        blk = at[:, bb * V:(bb + 1) * V]
                        nc.scalar.activation(blk, blk, AF.Exp,
                                             accum_out=zbuf[:, 2 * k + bb:
                                                            2 * k + bb + 1])
                else:
                    nc.scalar.dma_start(at[:, 0:V], acts[256 * NT:ROWS, :])
                    blk = at[:, 0:V]
                    nc.scalar.activation(blk, blk, AF.Exp,
                                         accum_out=zbuf[:, 2 * NT:2 * NT + 1])
            nc.scalar.dma_start(out_z[:], zbuf[:])

            # ------------- DP pipeline (DVE + sync-ring DMA) ------------
            for c in range(len(CHB) - 1):
                t0, tend = CHB[c], CHB[c + 1]
                if c < 2:
                    qt = qts[c]
                else:
                    qt = qp.tile([BS, CHMAX * QW], BF16, tag="q")
                    nc.sync.dma_start(qt[:, 0:(tend - t0) * QW],
                                      qq[:, t0 * QW:tend * QW])
                trange = range(max(t0, 1), tend)
                if c == 0:
                    nc.vector.tensor_copy(
                        wsb(), qt[:, 0:QW].rearrange("p (b s) -> p b s", b=2))
                for t in trange:
                    base = (t - t0) * QW
                    q01 = qt[:, base:base + QW].rearrange(
                        "p (b s) -> p b s", b=2)
                    nc.vector.tensor_add(t1[:], ws[:, 2:S + 2],
                                         ws[:, 1:S + 1])
                    nc.vector.tensor_add(t2[:], t1[:], ws[:, S + 2:2 * S + 2])
                    if t % K_RES == K_RES - 1:
                        j = t // K_RES
                        nc.vector.scalar_tensor_tensor(
                            wsb(), t2b(), 1.0, q01, ALU.mult, ALU.mult,
                            accum_out=cs[:, j:j + 1])
                        nc.vector.reciprocal(rbuf[:], cs[:, j:j + 1])
                    elif t % K_RES == 0:
                        acc = cs[:, NRES:NRES + 1] if t == T else None
                        nc.vector.scalar_tensor_tensor(
                            wsb(), t2b(), rbuf[:, 0:1], q01, ALU.mult,
                            ALU.mult, accum_out=acc)
                    else:
                        nc.vector.tensor_mul(wsb(), t2b(), q01)
            nc.sync.dma_start(out_cs[:], cs[:])
    nc.compile()
    return nc


_PROGRAM = None
_LAST_RESULTS = None


def _get_program():
    global _PROGRAM
    if _PROGRAM is None:
        _PROGRAM = _build_program()
    return _PROGRAM


def _host_prep(acts, ilen, labels, llen):
    """Per-core input maps plus host-side correction sums."""
    ext = np.zeros((B, S), np.int64)
    ext[:, 1::2] = labels
    m = np.zeros((B, S), np.float32)
    m[:, 2:] = ((ext[:, 2:] != 0) & (ext[:, 2:] != ext[:, :-2])).astype(
        np.float32)
    mtil = np.zeros((B, S), np.float32)
    mtil[:, :S - 2] = m[:, 2:]

    g = np.take_along_axis(acts, np.broadcast_to(ext[None], (T, B, S)), axis=2)
    gmax = g.max(axis=2).astype(np.float32) - BOOST          # [T, B]
    gt = (g - gmax[:, :, None]).astype(np.float32)           # [T, B, S]

    srange = np.arange(S)
    valid_s = srange[None, :] < (2 * llen + 1)[:, None]      # [B, S]
    gt = np.where(valid_s[None], gt, NEG)
    onehot = np.where(srange[None, :] == (2 * llen)[:, None],
                      np.float32(0.0), NEG)                  # [B, S]
    tmask = np.arange(T)[:, None] < ilen[None, :]            # [T, B]
    gt = np.where(tmask[:, :, None], gt, onehot[None])
    gt[0, :, 2:] = NEG
    gt_all = np.concatenate([gt, onehot[None]], axis=0)      # [T+1, B, S]

    q0 = np.exp(gt_all, dtype=np.float32)                    # [T+1, B, S]
    q1 = q0 * mtil[None]
    qarr = np.empty((B, T + 1, 2, S), np.float32)
    qarr[:, :, 0, :] = q0.transpose(1, 0, 2)
    qarr[:, :, 1, :] = q1.transpose(1, 0, 2)
    qq_full = qarr.reshape(B, QCOLS).astype(bf16)

    sum_gmax = (gmax.astype(np.float64) * tmask).sum(axis=0)  # [B]

    in_maps = []
    for c in range(NCORES):
        csl = slice(c * BS, (c + 1) * BS)
        acts_c = np.ascontiguousarray(
            acts[:, csl, :].reshape(ROWS, V).astype(np.float32))
        qq_c = np.ascontiguousarray(qq_full[csl])
        in_maps.append({"acts": acts_c, "qq": qq_c})
    return in_maps, sum_gmax


# z slot mapping: out_z[p, c] is Z for acts row r(p, c); t = r//8, u = p%8
_P = np.arange(128)
_RCOLS = np.empty((128, NZ), np.int64)
for _c in range(2 * NT):
    _RCOLS[:, _c] = 256 * (_c // 2) + 128 * (_c % 2) + _P
_RCOLS[:, 2 * NT] = 256 * NT + _P
_TCOLS = _RCOLS // BS          # [128, NZ]
_UCOLS = _P % BS               # [128]


def kernel(activations, input_lengths, labels, label_lengths):
    acts = np.asarray(activations, dtype=np.float32)
    ilen = np.asarray(input_lengths, dtype=np.int64)
    labs = np.asarray(labels, dtype=np.int64)
    llen = np.asarray(label_lengths, dtype=np.int64)

    in_maps, sum_gmax = _host_prep(acts, ilen, labs, llen)
    nc = _get_program()
    _r = run_bass_kernel_spmd(nc, in_maps, list(range(NCORES)))
    global _LAST_RESULTS
    _LAST_RESULTS = _r
    res = _r.results

    losses = np.zeros(B, np.float64)
    for c in range(NCORES):
        csl = slice(c * BS, (c + 1) * BS)
        lnz = np.log(res[c]["out_z"].astype(np.float64))     # [128, NZ]
        wmask = _TCOLS < ilen[csl][_UCOLS][:, None]          # [128, NZ]
        slz = np.zeros(BS)
        np.add.at(slz, _UCOLS.repeat(NZ),
                  (lnz * wmask).reshape(-1))
        ll = np.log(res[c]["out_cs"].astype(np.float64)).sum(axis=1)  # [BS]
        losses[csl] = -(ll + sum_gmax[csl] - slz)
    return np.float32(losses.mean())


# revision 10
# speedup vs baseline: 2.7163x; 1.0192x over previous
"""CTC loss on 8 Trainium2 cores.

Strategy (data-parallel over batch, B=64 -> 8 utterances/core):
  Device per core, two concurrent pipelines:
    - Streaming (ACT + HWDGE-DMA): acts [3200, 5000] f32 in 13 big tiles
      [128, 2, 5000]; ScalarE exp with accum_out -> Z per (t,u) row.
      ~64MB/core, memory-bound.
    - CTC DP on DVE only, layout [8 utterances (partitions), 101 ext states
      (free)]: shifts are free-axis offset views with zero guard columns;
      per step 3 DVE ops (add, add, dup-mul via broadcast+2-block write)
      in bf16 (2x DVE mode). Skip mask folded into a second emission table
      q1 = q0*mask; emissions q = exp(gtilde) precomputed host-side in
      bf16, streamed in chunks (first two small chunks ride the ACT HWDGE
      ring ahead of the acts tiles for a fast DP start; the rest use the
      sync-engine ring). Rescale by sum every 16 steps via
      scalar_tensor_tensor with a per-partition reciprocal; the STT accum
      also logs the rescale constants c into cs.
    - Freeze (t >= input_len) and final readout folded into one-hot q
      columns; final step's STT accum gives the end-state mass directly.
  Device outputs are raw Z [128, 25] and cs [8, 26]; the host does the
  tiny ln + masked reductions, corrections sum(gmax) - sum(logZ), mean.
"""
import numpy as np
import ml_dtypes

import concourse.bass as bass
import concourse.bacc as bacc
import concourse.mybir as mybir
import concourse.tile as tile
from concourse.bass_utils import run_bass_kernel_spmd

T, B, V, L = 400, 64, 5000, 50
S = 2 * L + 1            # 101
NCORES = 8
BS = B // NCORES         # 8
ROWS = T * BS            # 3200
NT = 12                  # full [128, 2, V] tiles; + one [128, V] tail tile
NZ = 2 * NT + 1          # 25 Z slots
BOOST = np.float32(1.5)
K_RES = 32
NRES = T // K_RES        # 12 rescales; cs has NRES+1 slots
QW = 2 * S               # 202
QCOLS = (T + 1) * QW     # 81002
# q chunk boundaries in t: small first chunks for a fast DP start
CHB = [0, 5, 12, 22, 37, 57, 87, 127, 177, 227, 277, 327, 377, T + 1]
CHMAX = max(b - a for a, b in zip(CHB, CHB[1:]))
NEG = np.float32(-10000.0)
F32 = mybir.dt.float32
BF16 = mybir.dt.bfloat16
bf16 = ml_dtypes.bfloat16
AF = mybir.ActivationFunctionType
ALU = mybir.AluOpType


def _build_program():
    nc = bacc.Bacc(None, target_bir_lowering=False)
    acts = nc.dram_tensor("acts", [ROWS, V], F32, kind="ExternalInput")
    qq = nc.dram_tensor("qq", [BS, QCOLS], BF16, kind="ExternalInput")
    out_z = nc.dram_tensor("out_z", [128, NZ], F32, kind="ExternalOutput")
    out_cs = nc.dram_tensor("out_cs", [BS, NRES + 1], F32,
                            kind="ExternalOutput")

    with tile.TileContext(nc) as tc:
        with (
            tc.tile_pool(name="mp", bufs=1) as mp,
            tc.tile_pool(name="ap", bufs=3) as ap,
            tc.tile_pool(name="qp", bufs=3) as qp,
        ):
            # ---------------- persistent state ----------------
            ws = mp.tile([BS, 2 * S + 4], BF16)   # [g g a(101) g g b(101)]
            nc.vector.memset(ws[:], 0.0)
            t1 = mp.tile([BS, S], BF16)
            t2 = mp.tile([BS, S], BF16)
            cs = mp.tile([BS, NRES + 1], F32)
            rbuf = mp.tile([BS, 1], F32)
            zbuf = mp.tile([128, NZ], F32)

            def wsb():   # state write view: cols {2..102, 105..205}
                return ws[:].rearrange("p (b s) -> p b s", b=2)[:, :, 2:S + 2]

            def t2b():   # t2 broadcast over the two state blocks
                return t2[:].unsqueeze(1).broadcast_to((BS, 2, S))

            # ------------- streaming pipeline (ACT engine + ring) -------
            # First two (small) q chunks ride the ACT HWDGE ring ahead of
            # the 5MB acts tiles, so the DP starts within a few us.
            qts = {}
            for c in range(4):
                qte = qp.tile([BS, CHMAX * QW], BF16, tag="q")
                qts[c] = qte
                nc.scalar.dma_start(
                    qte[:, 0:(CHB[c + 1] - CHB[c]) * QW],
                    qq[:, CHB[c] * QW:CHB[c + 1] * QW])
            for k in range(NT + 1):
                at = ap.tile([128, 2 * V], F32, tag="acts")
                if k < NT:
                    src = acts[256 * k:256 * (k + 1), :].rearrange(
                        "(b p) v -> p b v", b=2)
                    nc.scalar.dma_start(at[:], src)
                    for bb in range(2):
                        blk = at[:, bb * V:(bb + 1) * V]
                        nc.scalar.activation(blk, blk, AF.Exp,
                                             accum_out=zbuf[:, 2 * k + bb:
                                                            2 * k + bb + 1])
                else:
                    nc.scalar.dma_start(at[:, 0:V], acts[256 * NT:ROWS, :])
                    blk = at[:, 0:V]
                    nc.scalar.activation(blk, blk, AF.Exp,
                                         accum_out=zbuf[:, 2 * NT:2 * NT + 1])
            nc.scalar.dma_start(out_z[:], zbuf[:])

            # ------------- DP pipeline (DVE + sync-ring DMA) ------------
            for c in range(len(CHB) - 1):
                t0, tend = CHB[c], CHB[c + 1]
                if c < 4:
                    qt = qts[c]
                else:
                    qt = qp.tile([BS, CHMAX * QW], BF16, tag="q")
                    nc.sync.dma_start(qt[:, 0:(tend - t0) * QW],
                                      qq[:, t0 * QW:tend * QW])
                trange = range(max(t0, 1), tend)
                if c == 0:
                    nc.vector.tensor_copy(
                        wsb(), qt[:, 0:QW].rearrange("p (b s) -> p b s", b=2))
                for t in trange:
                    base = (t - t0) * QW
                    w = min(2 * t + 2, S)   # active-state prefix band
                    q01 = qt[:, base:base + QW].rearrange(
                        "p (b s) -> p b s", b=2)[:, :, 0:w]
                    wsv = wsb()[:, :, 0:w]
                    t2v = t2[:, 0:w].unsqueeze(1).broadcast_to((BS, 2, w))
                    nc.vector.tensor_add(t1[:, 0:w], ws[:, 2:w + 2],
                                         ws[:, 1:w + 1])
                    nc.vector.tensor_add(t2[:, 0:w], t1[:, 0:w],
                                         ws[:, S + 2:S + 2 + w])
                    if t % K_RES == K_RES - 1:
                        j = t // K_RES
                        nc.vector.scalar_tensor_tensor(
                            wsv, t2v, 1.0, q01, ALU.mult, ALU.mult,
                            accum_out=cs[:, j:j + 1])
                        nc.vector.reciprocal(rbuf[:], cs[:, j:j + 1])
                    elif t % K_RES == 0:
                        nc.vector.scalar_tensor_tensor(
                            wsv, t2v, rbuf[:, 0:1], q01, ALU.mult, ALU.mult)
                    elif t == T:
                        nc.vector.scalar_tensor_tensor(
                            wsv, t2v, 1.0, q01, ALU.mult, ALU.mult,
                            accum_out=cs[:, NRES:NRES + 1])
                    else:
                        nc.vector.tensor_mul(wsv, t2v, q01)
            nc.sync.dma_start(out_cs[:], cs[:])
    nc.compile()
    return nc


_PROGRAM = None
_LAST_RESULTS = None


def _get_program():
    global _PROGRAM
    if _PROGRAM is None:
        _PROGRAM = _build_program()
    return _PROGRAM


def _host_prep(acts, ilen, labels, llen):
    """Per-core input maps plus host-side correction sums."""
    ext = np.zeros((B, S), np.int64)
    ext[:, 1::2] = labels
    m = np.zeros((B, S), np.float32)
    m[:, 2:] = ((ext[:, 2:] != 0) & (ext[:, 2:] != ext[:, :-2])).astype(
        np.float32)
    mtil = np.zeros((B, S), np.float32)
    mtil[:, :S - 2] = m[:, 2:]

    g = np.take_along_axis(acts, np.broadcast_to(ext[None], (T, B, S)), axis=2)
    gmax = g.max(axis=2).astype(np.float32) - BOOST          # [T, B]
    gt = (g - gmax[:, :, None]).astype(np.float32)           # [T, B, S]

    srange = np.arange(S)
    valid_s = srange[None, :] < (2 * llen + 1)[:, None]      # [B, S]
    gt = np.where(valid_s[None], gt, NEG)
    onehot = np.where(srange[None, :] == (2 * llen)[:, None],
                      np.float32(0.0), NEG)                  # [B, S]
    tmask = np.arange(T)[:, None] < ilen[None, :]            # [T, B]
    gt = np.where(tmask[:, :, None], gt, onehot[None])
    gt[0, :, 2:] = NEG
    gt_all = np.concatenate([gt, onehot[None]], axis=0)      # [T+1, B, S]

    q0 = np.exp(gt_all, dtype=np.float32)                    # [T+1, B, S]
    q1 = q0 * mtil[None]
    qarr = np.empty((B, T + 1, 2, S), np.float32)
    qarr[:, :, 0, :] = q0.transpose(1, 0, 2)
    qarr[:, :, 1, :] = q1.transpose(1, 0, 2)
    qq_full = qarr.reshape(B, QCOLS).astype(bf16)

    sum_gmax = (gmax.astype(np.float64) * tmask).sum(axis=0)  # [B]

    in_maps = []
    for c in range(NCORES):
        csl = slice(c * BS, (c + 1) * BS)
        acts_c = np.ascontiguousarray(
            acts[:, csl, :].reshape(ROWS, V).astype(np.float32))
        qq_c = np.ascontiguousarray(qq_full[csl])
        in_maps.append({"acts": acts_c, "qq": qq_c})
    return in_maps, sum_gmax


# z slot mapping: out_z[p, c] is Z for acts row r(p, c); t = r//8, u = p%8
_P = np.arange(128)
_RCOLS = np.empty((128, NZ), np.int64)
for _c in range(2 * NT):
    _RCOLS[:, _c] = 256 * (_c // 2) + 128 * (_c % 2) + _P
_RCOLS[:, 2 * NT] = 256 * NT + _P
_TCOLS = _RCOLS // BS          # [128, NZ]
_UCOLS = _P % BS               # [128]


def kernel(activations, input_lengths, labels, label_lengths):
    acts = np.asarray(activations, dtype=np.float32)
    ilen = np.asarray(input_lengths, dtype=np.int64)
    labs = np.asarray(labels, dtype=np.int64)
    llen = np.asarray(label_lengths, dtype=np.int64)

    in_maps, sum_gmax = _host_prep(acts, ilen, labs, llen)
    nc = _get_program()
    _r = run_bass_kernel_spmd(nc, in_maps, list(range(NCORES)))
    global _LAST_RESULTS
    _LAST_RESULTS = _r
    res = _r.results

    losses = np.zeros(B, np.float64)
    for c in range(NCORES):
        csl = slice(c * BS, (c + 1) * BS)
        lnz = np.log(res[c]["out_z"].astype(np.float64))     # [128, NZ]
        wmask = _TCOLS < ilen[csl][_UCOLS][:, None]          # [128, NZ]
        slz = np.zeros(BS)
        np.add.at(slz, _UCOLS.repeat(NZ),
                  (lnz * wmask).reshape(-1))
        ll = np.log(res[c]["out_cs"].astype(np.float64)).sum(axis=1)  # [BS]
        losses[csl] = -(ll + sum_gmax[csl] - slz)
    return np.float32(losses.mean())


# revision 11
# speedup vs baseline: 2.7514x; 1.0129x over previous
"""CTC loss on 8 Trainium2 cores.

Strategy (data-parallel over batch, B=64 -> 8 utterances/core):
  Device per core, two concurrent pipelines:
    - Streaming (ACT + HWDGE-DMA): acts [3200, 5000] f32 in 13 big tiles
      [128, 2, 5000]; ScalarE exp with accum_out -> Z per (t,u) row.
      ~64MB/core, memory-bound.
    - CTC DP on DVE only, layout [8 utterances (partitions), 101 ext states
      (free)]: shifts are free-axis offset views with zero guard columns;
      per step 3 DVE ops (add, add, dup-mul via broadcast+2-block write)
      in bf16 (2x DVE mode). Skip mask folded into a second emission table
      q1 = q0*mask; emissions q = exp(gtilde) precomputed host-side in
      bf16, streamed in chunks (first two small chunks ride the ACT HWDGE
      ring ahead of the acts tiles for a fast DP start; the rest use the
      sync-engine ring). Rescale by sum every 16 steps via
      scalar_tensor_tensor with a per-partition reciprocal; the STT accum
      also logs the rescale constants c into cs.
    - Freeze (t >= input_len) and final readout folded into one-hot q
      columns; final step's STT accum gives the end-state mass directly.
  Device outputs are raw Z [128, 25] and cs [8, 26]; the host does the
  tiny ln + masked reductions, corrections sum(gmax) - sum(logZ), mean.
"""
import numpy as np
import ml_dtypes

import concourse.bass as bass
import concourse.bacc as bacc
import concourse.mybir as mybir
import concourse.tile as tile
from concourse.bass_utils import run_bass_kernel_spmd

T, B, V, L = 400, 64, 5000, 50
S = 2 * L + 1            # 101
NCORES = 8
BS = B // NCORES         # 8
ROWS = T * BS            # 3200
NT = 12                  # full [128, 2, V] tiles; + one [128, V] tail tile
NZ = 2 * NT + 1          # 25 Z slots
BOOST = np.float32(1.5)
K_RES = 32
NRES = T // K_RES        # 12 rescales; cs has NRES+1 slots
QW = 2 * S               # 202
QCOLS = (T + 1) * QW     # 81002
# q chunk boundaries in t: small first chunks for a fast DP start
CHB = [0, 5, 12, 22, 37, 57, 87, 127, 177, 227, 277, 327, 377, T + 1]
CHMAX = max(b - a for a, b in zip(CHB, CHB[1:]))
NEG = np.float32(-10000.0)
F32 = mybir.dt.float32
BF16 = mybir.dt.bfloat16
bf16 = ml_dtypes.bfloat16
AF = mybir.ActivationFunctionType
ALU = mybir.AluOpType


def _build_program():
    nc = bacc.Bacc(None, target_bir_lowering=False)
    acts = nc.dram_tensor("acts", [ROWS, V], F32, kind="ExternalInput")
    qq = nc.dram_tensor("qq", [BS, QCOLS], BF16, kind="ExternalInput")
    out_z = nc.dram_tensor("out_z", [128, NZ], F32, kind="ExternalOutput")
    out_cs = nc.dram_tensor("out_cs", [BS, NRES + 1], F32,
                            kind="ExternalOutput")

    with tile.TileContext(nc) as tc:
        with (
            tc.tile_pool(name="mp", bufs=1) as mp,
            tc.tile_pool(name="ap", bufs=2) as ap,
            tc.tile_pool(name="qp", bufs=5) as qp,
        ):
            # ---------------- persistent state ----------------
            ws = mp.tile([BS, 2 * S + 4], BF16)   # [g g a(101) g g b(101)]
            nc.vector.memset(ws[:], 0.0)
            t1 = mp.tile([BS, S], BF16)
            t2 = mp.tile([BS, S], BF16)
            cs = mp.tile([BS, NRES + 1], F32)
            rbuf = mp.tile([BS, 1], F32)
            zbuf = mp.tile([128, NZ], F32)

            def wsb():   # state write view: cols {2..102, 105..205}
                return ws[:].rearrange("p (b s) -> p b s", b=2)[:, :, 2:S + 2]

            def t2b():   # t2 broadcast over the two state blocks
                return t2[:].unsqueeze(1).broadcast_to((BS, 2, S))

            # ------------- streaming pipeline (ACT engine + ring) -------
            # First two (small) q chunks ride the ACT HWDGE ring ahead of
            # the 5MB acts tiles, so the DP starts within a few us.
            qts = {}
            for c in range(4):
                qte = qp.tile([BS, CHMAX * QW], BF16, tag="q")
                qts[c] = qte
                nc.scalar.dma_start(
                    qte[:, 0:(CHB[c + 1] - CHB[c]) * QW],
                    qq[:, CHB[c] * QW:CHB[c + 1] * QW])
            for k in range(NT + 1):
                at = ap.tile([128, 2 * V], F32, tag="acts")
                if k < NT:
                    src = acts[256 * k:256 * (k + 1), :].rearrange(
                        "(b p) v -> p b v", b=2)
                    nc.scalar.dma_start(at[:], src)
                    for bb in range(2):
                        blk = at[:, bb * V:(bb + 1) * V]
                        nc.scalar.activation(blk, blk, AF.Exp,
                                             accum_out=zbuf[:, 2 * k + bb:
                                                            2 * k + bb + 1])
                else:
                    nc.scalar.dma_start(at[:, 0:V], acts[256 * NT:ROWS, :])
                    blk = at[:, 0:V]
                    nc.scalar.activation(blk, blk, AF.Exp,
                                         accum_out=zbuf[:, 2 * NT:2 * NT + 1])
            nc.scalar.dma_start(out_z[:], zbuf[:])

            # ------------- DP pipeline (DVE + sync-ring DMA) ------------
            for c in range(len(CHB) - 1):
                t0, tend = CHB[c], CHB[c + 1]
                if c < 4:
                    qt = qts[c]
                else:
                    qt = qp.tile([BS, CHMAX * QW], BF16, tag="q")
                    nc.sync.dma_start(qt[:, 0:(tend - t0) * QW],
                                      qq[:, t0 * QW:tend * QW])
                trange = range(max(t0, 1), tend)
                if c == 0:
                    nc.vector.tensor_copy(
                        wsb(), qt[:, 0:QW].rearrange("p (b s) -> p b s", b=2))
                for t in trange:
                    base = (t - t0) * QW
                    w = min(2 * t + 2, S)   # active-state prefix band
                    q01 = qt[:, base:base + QW].rearrange(
                        "p (b s) -> p b s", b=2)[:, :, 0:w]
                    wsv = wsb()[:, :, 0:w]
                    t2v = t2[:, 0:w].unsqueeze(1).broadcast_to((BS, 2, w))
                    nc.vector.tensor_add(t1[:, 0:w], ws[:, 2:w + 2],
                                         ws[:, 1:w + 1])
                    nc.vector.tensor_add(t2[:, 0:w], t1[:, 0:w],
                                         ws[:, S + 2:S + 2 + w])
                    if t % K_RES == K_RES - 1:
                        j = t // K_RES
                        nc.vector.scalar_tensor_tensor(
                            wsv, t2v, 1.0, q01, ALU.mult, ALU.mult,
                            accum_out=cs[:, j:j + 1])
                        nc.vector.reciprocal(rbuf[:], cs[:, j:j + 1])
                    elif t % K_RES == 0:
                        nc.vector.scalar_tensor_tensor(
                            wsv, t2v, rbuf[:, 0:1], q01, ALU.mult, ALU.mult)
                    elif t == T:
                        nc.vector.scalar_tensor_tensor(
                            wsv, t2v, 1.0, q01, ALU.mult, ALU.mult,
                            accum_out=cs[:, NRES:NRES + 1])
                    else:
                        nc.vector.tensor_mul(wsv, t2v, q01)
            nc.sync.dma_start(out_cs[:], cs[:])
    nc.compile()
    return nc


_PROGRAM = None
_LAST_RESULTS = None


def _get_program():
    global _PROGRAM
    if _PROGRAM is None:
        _PROGRAM = _build_program()
    return _PROGRAM


def _host_prep(acts, ilen, labels, llen):
    """Per-core input maps plus host-side correction sums."""
    ext = np.zeros((B, S), np.int64)
    ext[:, 1::2] = labels
    m = np.zeros((B, S), np.float32)
    m[:, 2:] = ((ext[:, 2:] != 0) & (ext[:, 2:] != ext[:, :-2])).astype(
        np.float32)
    mtil = np.zeros((B, S), np.float32)
    mtil[:, :S - 2] = m[:, 2:]

    g = np.take_along_axis(acts, np.broadcast_to(ext[None], (T, B, S)), axis=2)
    gmax = g.max(axis=2).astype(np.float32) - BOOST          # [T, B]
    gt = (g - gmax[:, :, None]).astype(np.float32)           # [T, B, S]

    srange = np.arange(S)
    valid_s = srange[None, :] < (2 * llen + 1)[:, None]      # [B, S]
    gt = np.where(valid_s[None], gt, NEG)
    onehot = np.where(srange[None, :] == (2 * llen)[:, None],
                      np.float32(0.0), NEG)                  # [B, S]
    tmask = np.arange(T)[:, None] < ilen[None, :]            # [T, B]
    gt = np.where(tmask[:, :, None], gt, onehot[None])
    gt[0, :, 2:] = NEG
    gt_all = np.concatenate([gt, onehot[None]], axis=0)      # [T+1, B, S]

    q0 = np.exp(gt_all, dtype=np.float32)                    # [T+1, B, S]
    q1 = q0 * mtil[None]
    qarr = np.empty((B, T + 1, 2, S), np.float32)
    qarr[:, :, 0, :] = q0.transpose(1, 0, 2)
    qarr[:, :, 1, :] = q1.transpose(1, 0, 2)
    qq_full = qarr.reshape(B, QCOLS).astype(bf16)

    sum_gmax = (gmax.astype(np.float64) * tmask).sum(axis=0)  # [B]

    in_maps = []
    for c in range(NCORES):
        csl = slice(c * BS, (c + 1) * BS)
        acts_c = np.ascontiguousarray(
            acts[:, csl, :].reshape(ROWS, V).astype(np.float32))
        qq_c = np.ascontiguousarray(qq_full[csl])
        in_maps.append({"acts": acts_c, "qq": qq_c})
    return in_maps, sum_gmax


# z slot mapping: out_z[p, c] is Z for acts row r(p, c); t = r//8, u = p%8
_P = np.arange(128)
_RCOLS = np.empty((128, NZ), np.int64)
for _c in range(2 * NT):
    _RCOLS[:, _c] = 256 * (_c // 2) + 128 * (_c % 2) + _P
_RCOLS[:, 2 * NT] = 256 * NT + _P
_TCOLS = _RCOLS // BS          # [128, NZ]
_UCOLS = _P % BS               # [128]


def kernel(activations, input_lengths, labels, label_lengths):
    acts = np.asarray(activations, dtype=np.float32)
    ilen = np.asarray(input_lengths, dtype=np.int64)
    labs = np.asarray(labels, dtype=np.int64)
    llen = np.asarray(label_lengths, dtype=np.int64)

    in_maps, sum_gmax = _host_prep(acts, ilen, labs, llen)
    nc = _get_program()
    _r = run_bass_kernel_spmd(nc, in_maps, list(range(NCORES)))
    global _LAST_RESULTS
    _LAST_RESULTS = _r
    res = _r.results

    losses = np.zeros(B, np.float64)
    for c in range(NCORES):
        csl = slice(c * BS, (c + 1) * BS)
        lnz = np.log(res[c]["out_z"].astype(np.float64))     # [128, NZ]
        wmask = _TCOLS < ilen[csl][_UCOLS][:, None]          # [128, NZ]
        slz = np.zeros(BS)
        np.add.at(slz, _UCOLS.repeat(NZ),
                  (lnz * wmask).reshape(-1))
        ll = np.log(res[c]["out_cs"].astype(np.float64)).sum(axis=1)  # [BS]
        losses[csl] = -(ll + sum_gmax[csl] - slz)
    return np.float32(losses.mean())
